# revision 51
# baseline (speedup 1.0000x reference)
"""MQA (GQA, 1 KV group) attention kernel for 8 Trainium2 NeuronCores.

Sharding: core c -> batch b = c//4, head-group hg = c%4 (4 of 16 query heads).
Each core computes Q/K/V projections from x[b]^T, causal attention for its 4
heads in transposed layout, and a partial output projection.  Host sums the 4
partials per batch and adds bo.

Schedule is built to keep the PE streaming at its max p-state:
 - attention q-chunks of 128 rows; per kv-tile ONE 4-head-wide scores matmul
   [128kv x 512(h,q)], ONE exp activation, ONE AV matmul, and ONE fused
   rowsum+broadcast matmul (ones^T @ es accumulated in PSUM) -- 3 PE + 1 ACT
   instructions per kv tile, all 512 free columns.
 - causal diag handled by a multiplicative 0/1 bf16 mask on DVE; padding mask
   enters as a per-kv-partition bias in the exp activation.
 - normalization: reciprocal_approx_fast + one DVE mul per chunk.
 - stage-1 bias adds and stage-3 PSUM drains on DVE; exp one-kt-ahead
   software pipeline; outproj(qc-1) interleaved after attn(qc) so the PE has
   filler work while the scalar engine catches up.
"""

import sys

sys.path.insert(0, "/opt/trn_rl_repo")

import ml_dtypes
import numpy as np

import concourse.bass as bass
import concourse.tile as tile
from concourse import bacc
from concourse import mybir
from concourse.bass import ts
from concourse.bass_utils import run_bass_kernel_spmd
from concourse.masks import make_identity

B, S, HID = 2, 2048, 2048
H, D = 16, 128
HPC = 4              # heads per core
DPH = HPC * D        # 512
NCORES = 8
SC1 = 512            # stage-1 s-chunk
NSC1 = S // SC1      # 4
QC = 128             # attention q-chunk
NQC = S // QC        # 16
NT = S // 128        # 16
NHT = HID // 128     # 16
SCALE = 1.0 / float(np.sqrt(D))
NEG = -1.0e9

F32 = mybir.dt.float32
BF16 = mybir.dt.bfloat16
NP_BF16 = ml_dtypes.bfloat16

_PROGRAM = None
LAST_RESULT = None


def _build_program():
    nc = bacc.Bacc()
    # all big inputs pre-shuffled on host so each DMA reads long contiguous
    # per-partition lines (16KB) instead of 1KB strided rows
    xT = nc.declare_dram_parameter("xT", [128, NSC1, NHT, SC1], BF16, isOutput=False)
    wq = nc.declare_dram_parameter("wq", [128, NHT, DPH], BF16, isOutput=False)
    wk = nc.declare_dram_parameter("wk", [128, NHT, D], BF16, isOutput=False)
    wv = nc.declare_dram_parameter("wv", [128, NHT, D], BF16, isOutput=False)
    wo = nc.declare_dram_parameter("wo", [128, HPC, HID], BF16, isOutput=False)
    bq = nc.declare_dram_parameter("bq", [128, HPC], F32, isOutput=False)
    bkv = nc.declare_dram_parameter("bkv", [128, 2], F32, isOutput=False)
    padb = nc.declare_dram_parameter("padb", [128, NT], F32, isOutput=False)
    mask4 = nc.declare_dram_parameter("mask4", [128, HPC, QC], BF16, isOutput=False)
    # bf16 partial outputs: host sums 4 partials per batch in f32; the extra
    # ~0.2% fro error is well within the 2e-2 budget and halves output DMA
    out = nc.declare_dram_parameter("out", [S, HID], BF16, isOutput=True)

    Exp = mybir.ActivationFunctionType.Exp

    with tile.TileContext(nc) as tc:
        with (
            tc.tile_pool(name="consts", bufs=1) as consts,
            tc.tile_pool(name="persist", bufs=1) as persist,
            tc.tile_pool(name="esb", bufs=1) as esb,
            tc.tile_pool(name="ps", bufs=1, space="PSUM") as ps,
        ):
            # ---- DMA issue is spread across engine DGE queues so the Sync
            # engine's serial ~0.7us-per-issue cost doesn't delay startup ----
            # wk first on the fast Sync queue: it gates the first matmul;
            # wv is not needed until the V group, so it rides behind the
            # first x^T sub-chunks
            wk_sb = consts.tile([128, NHT, D], BF16)
            xts = persist.tile([128, NSC1, NHT, SC1], BF16)
            wv_sb = consts.tile([128, NHT, D], BF16)
            # interleave wk quarters with sc0 sub-chunks: the K projection
            # consumes (wk[ht], x[ht]) in ht order, so the first matmul only
            # needs the first 1/4 of each
            for t4 in range(4):
                nc.sync.dma_start(
                    wk_sb[:, 4 * t4 : 4 * t4 + 4, :], wk[:, 4 * t4 : 4 * t4 + 4, :]
                )
                nc.sync.dma_start(
                    xts[:, 0, 4 * t4 : 4 * t4 + 4, :],
                    xT[:, 0, 4 * t4 : 4 * t4 + 4, :],
                )
                if t4 == 1:
                    nc.sync.dma_start(wv_sb[:], wv[:])
            wq_sb = persist.tile([128, NHT, DPH], BF16)
            nc.sync.dma_start(wq_sb[:, 0:8], wq[:, 0:8])
            nc.sync.dma_start(wq_sb[:, 8:16], wq[:, 8:16])
            wo_sb = persist.tile([128, HPC, HID], BF16)
            nc.sync.dma_start(wo_sb[:], wo[:])
            for sc in range(1, NSC1):
                nc.sync.dma_start(xts[:, sc], xT[:, sc])

            # small/early params on other engines' queues
            bq_sb = consts.tile([128, HPC], F32)
            nc.scalar.dma_start(bq_sb[:], bq[:])
            bkv_sb = consts.tile([128, 2], F32)
            nc.scalar.dma_start(bkv_sb[:], bkv[:])
            padb_sb = consts.tile([128, NT], F32)
            nc.scalar.dma_start(padb_sb[:], padb[:])
            mask_sb = consts.tile([128, HPC, QC], BF16)
            nc.scalar.dma_start(mask_sb[:], mask4[:])
            ident = consts.tile([128, 128], BF16)
            make_identity(nc, ident[:])
            ones128 = consts.tile([128, 128], BF16)
            nc.vector.memset(ones128[:], 1.0)

            # ---- persistent activations ----
            KT = persist.tile([128, S], BF16)         # K^T [d, kv]
            V = persist.tile([128, NT, 128], BF16)    # V tiles [kv_p, kt, d]
            QT = persist.tile([128, HPC, S], BF16)    # Q^T [d, h, q]
            OT = persist.tile([128, HPC, S], BF16)    # normalized (exp S)V ^T

            def stage1(sc, q_first):
                def kgroup():
                    psk = ps.tile([128, SC1], F32, tag="bg", bufs=2, name="psk")
                    for ht in range(NHT):
                        nc.tensor.matmul(
                            psk[:], wk_sb[:, ht, :], xts[:, sc, ht, :],
                            start=(ht == 0), stop=(ht == NHT - 1),
                        )
                    nc.vector.tensor_scalar_add(
                        KT[:, ts(sc, SC1)], psk[:], bkv_sb[:, 0:1]
                    )

                def vgroup():
                    psv = ps.tile([128, SC1], F32, tag="bg", bufs=2, name="psv")
                    for ht in range(NHT):
                        nc.tensor.matmul(
                            psv[:], wv_sb[:, ht, :], xts[:, sc, ht, :],
                            start=(ht == 0), stop=(ht == NHT - 1),
                        )
                    vt_s = esb.tile(
                        [128, SC1], BF16, tag="vt", bufs=2, name="vt"
                    )
                    nc.vector.tensor_scalar_add(vt_s[:], psv[:], bkv_sb[:, 1:2])
                    return vt_s

                def transposes(vt_s):
                    pstr = ps.tile(
                        [128, 4, 128], BF16, tag="bg", bufs=2, name="pstr"
                    )
                    for j in range(4):
                        nc.tensor.transpose(
                            pstr[:, j, :], vt_s[:, ts(j, 128)], ident[:]
                        )
                    nc.scalar.copy(V[:, 4 * sc : 4 * sc + 4, :], pstr[:])

                def qhead(dt):
                    psq = ps.tile(
                        [128, SC1], F32, tag="bg", bufs=2, name=f"psq{dt}"
                    )
                    for ht in range(NHT):
                        nc.tensor.matmul(
                            psq[:], wq_sb[:, ht, ts(dt, 128)],
                            xts[:, sc, ht, :],
                            start=(ht == 0), stop=(ht == NHT - 1),
                        )
                    nc.vector.tensor_scalar_add(
                        QT[:, dt, ts(sc, SC1)], psq[:],
                        bq_sb[:, dt : dt + 1],
                    )

                if not q_first:
                    # startup order: K/V first (small weights arrive first)
                    kgroup()
                    vt = vgroup()
                    qhead(0)
                    qhead(1)
                    transposes(vt)
                    qhead(2)
                    qhead(3)
                else:
                    # steady state: Q heads first — the next attention chunk
                    # needs QT immediately but this chunk's K/V tiles only
                    # several kv-iterations in, so the K/V drains are covered
                    qhead(0)
                    qhead(1)
                    qhead(2)
                    qhead(3)
                    kgroup()
                    vt = vgroup()
                    op_pop(1)
                    transposes(vt)
                    op_pop(1)

            # outproj work is queued as closures and woven between attention
            # kv-tiles, so the PE absorbs the scores->exp latency with real
            # work instead of idling (the attn phase alone is exp-paced)
            op_queue = []
            # flipped once all exps are done: scalar is then free to drain
            drain_scalar = [False]

            def outproj_enqueue(qc):
                ot = esb.tile([128, HID], BF16, tag="out", bufs=2, name="ot")

                def group(hc):
                    ps3 = ps.tile(
                        [128, SC1], F32, tag="bg", bufs=2, name=f"ps3_{hc}"
                    )
                    for dt in range(HPC):
                        nc.tensor.matmul(
                            ps3[:],
                            OT[:, dt, ts(qc, QC)],
                            wo_sb[:, dt, ts(hc, SC1)],
                            start=(dt == 0), stop=(dt == HPC - 1),
                        )
                    # drains stay off the scalar engine while exps still
                    # pace the attention loop; after the last exp the scalar
                    # engine is free and relieves the DVE backlog
                    if drain_scalar[0]:
                        nc.scalar.copy(ot[:, ts(hc, SC1)], ps3[:])
                    else:
                        nc.vector.tensor_scalar_add(
                            ot[:, ts(hc, SC1)], ps3[:], 0.0
                        )
                    if qc == NQC - 1:
                        # last row block: ship each quarter as it drains so
                        # the final transfer isn't serialized behind the
                        # whole block at kernel end
                        nc.sync.dma_start(
                            out[ts(qc, QC), ts(hc, SC1)], ot[:, ts(hc, SC1)]
                        )
                    elif hc == HID // SC1 - 1:
                        nc.sync.dma_start(out[ts(qc, QC), :], ot[:])

                for hc in range(HID // SC1):
                    op_queue.append(lambda h=hc: group(h))

            def op_pop(n):
                for _ in range(n):
                    if op_queue:
                        op_queue.pop(0)()

            def attn(qc):
                nkt = qc + 1
                pso = ps.tile([128, HPC, QC], F32, tag="o", bufs=2, name="pso")
                psr = ps.tile([128, HPC, QC], F32, tag="r", bufs=1, name="psr")
                pend = []
                tree = []  # binary-counter sum tree of es tiles, max level 2
                rsb_emitted = 0
                # rowsums: es tiles are tree-summed on DVE (bf16) up to
                # groups of 4, so the PE streams each quad once, not 4x
                # (level-3/oct summing measured slightly worse: the deeper
                # DVE chains delay the chunk-end flush)
                nrsb = nkt // 4 + (nkt % 4) // 2 + (nkt % 2)

                def rsb(src):
                    nonlocal rsb_emitted
                    nc.tensor.matmul(
                        psr[:], ones128[:], src[:],
                        start=(rsb_emitted == 0), stop=(rsb_emitted == nrsb - 1),
                    )
                    rsb_emitted += 1

                def tree_add(es):
                    node = (0, es)
                    while tree and tree[-1][0] == node[0] and node[0] < 2:
                        lvl, other = tree.pop()
                        tag = ("esp", "esq")[lvl]
                        s = esb.tile(
                            [128, HPC, QC], BF16, tag=tag, bufs=2, name=tag
                        )
                        nc.vector.tensor_add(s[:], other[:], node[1][:])
                        node = (lvl + 1, s)
                    if node[0] == 2:
                        rsb(node[1])
                    else:
                        tree.append(node)

                def consume():
                    es, kt = pend.pop(0)
                    nc.tensor.matmul(
                        pso[:], V[:, kt, :], es[:],
                        start=(kt == 0), stop=(kt == nkt - 1),
                    )
                    tree_add(es)

                for kt in range(nkt):
                    psS = ps.tile(
                        [128, HPC, QC], F32, tag="s", bufs=3, name="psS"
                    )
                    nc.tensor.matmul(
                        psS[:], KT[:, ts(kt, 128)], QT[:, :, ts(qc, QC)],
                        start=True, stop=True,
                    )
                    es = esb.tile([128, HPC, QC], BF16, tag="es", bufs=4, name="es")
                    nc.scalar.activation(
                        es[:], psS[:], Exp,
                        bias=padb_sb[:, kt : kt + 1], scale=SCALE,
                    )
                    if kt == qc:
                        nc.vector.tensor_mul(es[:], es[:], mask_sb[:])
                    if len(pend) >= 2:
                        consume()
                    pend.append((es, kt))
                    if kt % 2 == 1:
                        op_pop(1)
                while pend:
                    consume()
                # PE filler while the DVE finishes the leftover tree adds
                # (extra filler at the last chunk covers its normalize)
                op_pop(2 if qc < NQC - 1 else 4)
                for _, leftover in tree:
                    rsb(leftover)
                tree.clear()
                # normalize: OT[:, h, qc block] = pso * 1/psr
                rec = esb.tile([128, HPC, QC], F32, tag="rec", bufs=2, name="rec")
                nc.vector.reciprocal_approx_fast(rec[:], psr[:])
                nc.vector.tensor_mul(OT[:, :, ts(qc, QC)], pso[:], rec[:])

            # ---------------- main schedule ----------------
            for sc in range(NSC1):
                stage1(sc, q_first=(sc > 0))
                if sc == 0:
                    op_pop(2)
                for qc in range(4 * sc, 4 * sc + 4):
                    attn(qc)
                    outproj_enqueue(qc)
            drain_scalar[0] = True
            while op_queue:
                op_pop(1)

    nc.compile()
    return nc


def _get_program():
    global _PROGRAM
    if _PROGRAM is None:
        _PROGRAM = _build_program()
    return _PROGRAM


def kernel(**inputs):
    global LAST_RESULT
    hs = np.ascontiguousarray(inputs["hidden_states"], dtype=np.float32)
    pad = np.ascontiguousarray(inputs["padding_mask"], dtype=np.float32)
    Wq = np.asarray(inputs["Wq"], dtype=np.float32)
    Wk = np.asarray(inputs["Wk"], dtype=np.float32)
    Wv = np.asarray(inputs["Wv"], dtype=np.float32)
    Wo = np.asarray(inputs["Wo"], dtype=np.float32)
    bq_v = np.asarray(inputs["bq"], dtype=np.float32)
    bk_v = np.asarray(inputs["bk"], dtype=np.float32)
    bv_v = np.asarray(inputs["bv"], dtype=np.float32)
    bo_v = np.asarray(inputs["bo"], dtype=np.float32)

    # x^T pre-shuffled to [p, sc, ht, c]: partition lines are 16KB contiguous
    xTs = [
        np.ascontiguousarray(
            hs[b].T.reshape(NHT, 128, NSC1, SC1).transpose(1, 2, 0, 3)
        ).astype(NP_BF16)
        for b in range(B)
    ]
    WqT = Wq.T  # [HID, HID]
    # [p, ht, d] shuffles
    WkT = np.ascontiguousarray(
        Wk.T.reshape(NHT, 128, D).transpose(1, 0, 2)
    ).astype(NP_BF16)
    WvT = np.ascontiguousarray(
        Wv.T.reshape(NHT, 128, D).transpose(1, 0, 2)
    ).astype(NP_BF16)
    WoT = Wo.T  # [HID, HID]

    # causal 0/1 mask for the diagonal tile, [128 kv, h, 128 q]
    p_i = np.arange(128)[:, None]
    q_i = np.arange(QC)[None, :]
    m = (q_i >= p_i).astype(np.float32)
    mask4 = np.ascontiguousarray(
        np.broadcast_to(m[:, None, :], (128, HPC, QC))
    ).astype(NP_BF16)

    padbs = [
        np.ascontiguousarray((NEG * pad[b]).reshape(NT, 128).T) for b in range(B)
    ]
    bqs = [
        np.ascontiguousarray(
            bq_v[hg * DPH : (hg + 1) * DPH].reshape(HPC, 128).T
        )
        for hg in range(HPC)
    ]
    bkv = np.ascontiguousarray(np.stack([bk_v, bv_v], axis=1))  # [128, 2]

    nc = _get_program()
    in_maps = []
    for c in range(NCORES):
        b, hg = c // 4, c % 4
        in_maps.append(
            {
                "xT": xTs[b],
                "wq": np.ascontiguousarray(
                    WqT[:, hg * DPH : (hg + 1) * DPH]
                    .reshape(NHT, 128, DPH)
                    .transpose(1, 0, 2)
                ).astype(NP_BF16),
                "wk": WkT,
                "wv": WvT,
                "wo": np.ascontiguousarray(
                    WoT[hg * DPH : (hg + 1) * DPH, :]
                    .reshape(HPC, 128, HID)
                    .transpose(1, 0, 2)
                ).astype(NP_BF16),
                "bq": bqs[hg],
                "bkv": bkv,
                "padb": padbs[b],
                "mask4": mask4,
            }
        )

    LAST_RESULT = run_bass_kernel_spmd(nc, in_maps, list(range(NCORES)))
    res = LAST_RESULT.results

    outp = np.zeros((B, S, HID), np.float32)
    for c in range(NCORES):
        outp[c // 4] += res[c]["out"]
    outp += bo_v[None, None, :]
    return outp


if __name__ == "__main__":
    rng = np.random.default_rng(0)
    demo = {
        "hidden_states": rng.standard_normal((B, S, HID), dtype=np.float32),
        "causal_mask": np.triu(np.ones((1, 1, S, S), np.float32), k=1),
        "padding_mask": np.zeros((B, S), np.float32),
        "Wq": (rng.standard_normal((HID, HID), dtype=np.float32) * 0.02),
        "bq": np.zeros((HID,), np.float32),
        "Wk": (rng.standard_normal((D, HID), dtype=np.float32) * 0.02),
        "bk": np.zeros((D,), np.float32),
        "Wv": (rng.standard_normal((D, HID), dtype=np.float32) * 0.02),
        "bv": np.zeros((D,), np.float32),
        "Wo": (rng.standard_normal((HID, HID), dtype=np.float32) * 0.02),
        "bo": np.zeros((HID,), np.float32),
    }
    o = kernel(**demo)
    print("kernel output", o.shape, o.dtype, float(np.abs(o).mean()))
